# revision 60
# baseline (speedup 1.0000x reference)
"""Trainium2 Bass kernel for NanoAttention (B=4, T=2048, C=1024, H=16, causal).

Sharding: 8 cores = 4 batches x 2 head-groups (8 heads each).
Per core (b, hg):
  - column-parallel qkv:  q,k produced in [channel, token] (transposed) layout,
    v produced in [token, channel] (natural) layout with an appended ones
    column per head (fuses the softmax denominator into the AV matmul).
  - causal attention per head with S^T in [key, query] layout; exp on ACT with
    scale=1/sqrt(D); block-causal skipping + triangular masking on the
    diagonal blocks.
  - row-parallel proj producing a partial output [C, T] in bf16; the host adds
    the two head-group partials per batch and transposes back.

v2 structural changes vs v1 (319us):
  - software-pipelined emission: the PE queue is in-order, so S(k+1) is
    emitted BEFORE AV(k) and independent dense matmuls (qkv of the next
    group, proj of the previous, v chains) are drip-fed between attention
    blocks to fill the PE during ACT(exp)-paced stretches.
  - startup: xt/wqk DMAs interleaved per contraction chunk so the first
    matmul starts after ~256KB instead of ~3MB; keeps HAM at 8/8.
  - dense psum chains double-buffered (2 bufs) to kill inter-chain stalls.
  - softmax reciprocal on DVE (reciprocal_approx_fast) instead of ACT Ln/Exp.
  - reciprocal broadcast via gpsimd partition_broadcast (no DRAM roundtrip).
  - output DMA in bf16 (host accumulates partials in f32).
"""
import os
import sys

sys.path.insert(0, '/opt/trn_rl_repo')

import numpy as np
import orjson

import concourse.bass as bass
import concourse.mybir as mybir
import concourse.tile as tile
from concourse.bass_utils import run_bass_kernel_spmd

# ---------------------------------------------------------------------------
# Workaround for this container's walrus build: it enforces the HW limit of
# one sync-wait per instruction (two for EventSemaphore), but Tile's sem
# assignment can emit more (kernel-tail Drain waits on every DMA queue used;
# HWDGE stores can pick up two queue waits). Split the overflow onto
# preceding pure-wait EventSemaphore instructions on the same engine at
# JSON-serialization time so every compile path is covered.
# ---------------------------------------------------------------------------


def _split_multi_waits(data):
    n_split = 0
    for func in data.get("functions", []):
        for blk in func.get("blocks", []):
            insts = blk.get("instructions")
            if not insts:
                continue
            out = []
            for inst in insts:
                si = inst.get("sync_info")
                waits = (si or {}).get("on_wait") or []
                cap = 2 if inst.get("opcode") == "EventSemaphore" else 1
                if len(waits) > cap and "engine" in inst:
                    extra = waits[:-cap]
                    si["on_wait"] = waits[-cap:]
                    for i in range(0, len(extra), 2):
                        n_split += 1
                        out.append({
                            "debug": inst.get("debug"),
                            "engine": inst["engine"],
                            "ins": [],
                            "outs": [],
                            "name": f"{inst['name']}_wsplit{n_split}",
                            "opcode": "EventSemaphore",
                            "sync_info": {"on_wait": extra[i:i + 2],
                                          "on_update": []},
                        })
                out.append(inst)
            blk["instructions"] = out
    return data


_orig_to_json_bytes = bass.Bass.to_json_bytes


def _patched_to_json_bytes(self):
    return orjson.dumps(_split_multi_waits(orjson.loads(_orig_to_json_bytes(self))))


bass.Bass.to_json_bytes = _patched_to_json_bytes

# ---------------------------------------------------------------------------

B, T, C = 4, 2048, 1024
N_HEAD, D = 16, 64
HLOC = 8          # heads per core
CLOC = HLOC * D   # 512 local qkv channels per core
QG = 512          # query-group width
NG = T // QG      # 4 query groups
KB = 128          # key-block width
F32R = mybir.dt.float32r
F32 = mybir.dt.float32
BF16 = mybir.dt.bfloat16
CDT = BF16
ADT = CDT
EXP = mybir.ActivationFunctionType.Exp
SCALE = 1.0 / np.sqrt(D)
# reciprocal broadcast path: "dma" (DRAM roundtrip broadcast) or "gpsimd"
# (partition_broadcast — does NOT compile in this container's walrus:
# "ISA wrong length", same for the custom-DVE reciprocal_approx_fast).
BCAST = os.environ.get("ATTN_BCAST", "dma")
# engine for the yt normalize multiplies: "gpsimd" keeps the DMA-broadcast
# wait out of the DVE queue; "dve" is the fallback.
MUL = os.environ.get("ATTN_MUL", "gpsimd")


def _build_body(nc, tc, ctx, xt, wqkt, wvt, wpt, tri, ot):
    p_wqk = ctx.enter_context(tc.tile_pool(name="wqk", bufs=4))
    p_wv = ctx.enter_context(tc.tile_pool(name="wv", bufs=1))
    p_wp = ctx.enter_context(tc.tile_pool(name="wp", bufs=1))
    p_xt0 = ctx.enter_context(tc.tile_pool(name="xt0", bufs=4))
    p_xt = ctx.enter_context(tc.tile_pool(name="xt", bufs=2))
    p_k = ctx.enter_context(tc.tile_pool(name="ksb", bufs=4))
    p_q = ctx.enter_context(tc.tile_pool(name="qsb", bufs=8))
    p_vp = ctx.enter_context(tc.tile_pool(name="vp", bufs=16))
    p_es = ctx.enter_context(tc.tile_pool(name="es", bufs=7))
    # all four groups' yt stay alive: proj is deferred to attn3/tail
    p_yt = ctx.enter_context(tc.tile_pool(name="yt", bufs=16))
    p_ost = ctx.enter_context(tc.tile_pool(name="ost", bufs=4))
    p_one = ctx.enter_context(tc.tile_pool(name="one", bufs=1))
    p_rec = ctx.enter_context(tc.tile_pool(name="rec", bufs=6))
    p_ysb = ctx.enter_context(tc.tile_pool(name="ysb", bufs=6))
    p_bc = ctx.enter_context(tc.tile_pool(name="bc", bufs=4))
    if BCAST == "dma":
        p_drb = ctx.enter_context(tc.tile_pool(name="drb", bufs=2, space="DRAM"))
    ps_mm = ctx.enter_context(tc.tile_pool(name="psmm", bufs=2, space="PSUM"))
    ps_s = ctx.enter_context(tc.tile_pool(name="pss", bufs=2, space="PSUM"))
    ps_y = ctx.enter_context(tc.tile_pool(name="psy", bufs=2, space="PSUM"))

    # ---- static state ----
    # Each dma_start costs ~630ns of serial descriptor-generation on the
    # sync engine, so inputs are loaded with FEW multi-chunk transfers:
    # contraction-chunk views are AP slices of [128, n, free] tiles.
    wqk_sb = [None] * 8   # views: wqk_sb[kc] = [128, 2*CLOC]
    wv_sb = [None] * 8
    wp_sb = []
    xt_g = [[None] * 8 for _ in range(NG)]
    tri_sb = p_one.tile([KB, KB], CDT, tag="tri")
    ones_sb = p_one.tile([33, 64], F32R, tag="ones")
    nc.vector.memset(ones_sb.bitcast(F32), 1.0)
    k_sb = [p_k.tile([128, T], ADT, tag="ksb", name=f"ksb{c}") for c in range(4)]
    q_gs = [[None] * 4 for _ in range(NG)]
    vp_sb = []        # grows to 16 tiles, 4 per group
    yt_gs = [None] * NG

    def dma_x(g, kcs, pool):
        # one dma_start covering contraction chunks kcs (list); the host
        # pre-tiles x as [NG, 128, 8, QG] so each partition's run is
        # contiguous (n*1KB descriptors instead of n*128)
        n = len(kcs)
        t = pool.tile([128, n, QG], CDT, tag="xt", name=f"xt{g}_{kcs[0]}")
        nc.sync.dma_start(out=t, in_=xt[g, :, kcs[0]:kcs[0] + n, :])
        for i, kc in enumerate(kcs):
            xt_g[g][kc] = t[:, i, :]

    def dma_wqk(kc0):
        # one dma_start per PAIR of contraction chunks (host layout
        # [4, 128, 2, 2*CLOC])
        t = p_wqk.tile([128, 2, 2 * CLOC], CDT, tag="wqk", name=f"wqk{kc0}")
        nc.sync.dma_start(out=t, in_=wqkt[kc0 // 2, :, :, :])
        wqk_sb[kc0] = t[:, 0, :]
        wqk_sb[kc0 + 1] = t[:, 1, :]

    # ---- dense-fill machinery: each fill item emits ONE FULL accumulation
    # chain (8 or 4 back-to-back matmuls + epilogue). Chain granularity
    # keeps the PE instruction stream uniform: scattering single matmuls
    # between attention ops costs a ~130ns restart per insertion. ----
    def qk_chain(g, m):
        # m in 0..7: 0..3 -> q chunks (hp=m), 4..7 -> k chunks (hp=m-4)
        def emit(g=g, m=m):
            ps = ps_mm.tile([128, QG], F32, tag="psmm", name=f"qk{g}_{m}")
            for kc in range(8):
                nc.tensor.matmul(ps, wqk_sb[kc][:, m * 128:(m + 1) * 128],
                                 xt_g[g][kc], start=kc == 0, stop=kc == 7,
                                 skip_group_check=True)
            if m < 4:
                qt = p_q.tile([128, QG], ADT, tag="qsb", name=f"q{g}_{m}")
                nc.vector.tensor_copy(out=qt, in_=ps)
                q_gs[g][m] = qt
            else:
                nc.vector.tensor_copy(
                    out=k_sb[m - 4][:, g * QG:(g + 1) * QG], in_=ps)
            chains_done.add((g, m))
        return emit

    def v_chain(g, tb):
        def emit(g=g, tb=tb):
            ps = ps_mm.tile([128, CLOC], F32, tag="psmm", name=f"v{g}_{tb}")
            for kc in range(8):
                nc.tensor.matmul(ps, xt_g[g][kc][:, tb * 128:(tb + 1) * 128],
                                 wv_sb[kc], start=kc == 0, stop=kc == 7,
                                 skip_group_check=True)
            vp = p_vp.tile([128, HLOC, 65], ADT, tag="vp", name=f"vp{g}_{tb}")
            nc.vector.memset(vp[:, :, 64:65], 1.0)
            nc.vector.tensor_copy(
                out=vp[:, :, 0:64], in_=ps.rearrange("p (h d) -> p h d", d=64))
            vp_sb.append(vp)
        return emit

    ost_pending = {}

    def proj_chain(g, m):
        def emit(g=g, m=m):
            ps = ps_mm.tile([128, QG], F32, tag="psmm", name=f"pj{g}_{m}")
            for c in range(4):
                nc.tensor.matmul(ps, wp_sb[c][:, m * 128:(m + 1) * 128],
                                 yt_gs[g][c], start=c == 0, stop=c == 3,
                                 skip_group_check=True)
            # pair the output stores: one dma_start per two m-chunks
            # (halves the serial descriptor-generation on the sync engine)
            if m % 2 == 0:
                ost = p_ost.tile([128, 2, QG], CDT, tag="ost",
                                 name=f"ost{g}_{m}")
                ost_pending[g] = ost
            else:
                ost = ost_pending[g]
            nc.vector.tensor_copy(out=ost[:, m % 2, :], in_=ps)
            if m % 2 == 1:
                nc.sync.dma_start(out=ot[g, :, m - 1:m + 1, :], in_=ost)
        return emit

    fill = []          # FIFO of pending dense chains
    chains_done = set()

    # ---- startup: interleaved xt0/wqk pair-DMAs ordered by first
    # consumption, wv early (v chains are the first fill); first chains
    # ASAP ----
    for kc0 in range(0, 8, 2):
        dma_x(0, [kc0, kc0 + 1], p_xt0)
        dma_wqk(kc0)
        if kc0 == 2:
            wv_t = p_wv.tile([128, 8, CLOC], CDT, tag="wv")
            nc.sync.dma_start(out=wv_t, in_=wvt[:, :, :])
            for kc in range(8):
                wv_sb[kc] = wv_t[:, kc, :]
    nc.sync.dma_start(out=tri_sb, in_=tri[:, :])

    with nc.named_scope("qkv0"):
        qk_chain(0, 0)()
        qk_chain(0, 4)()

    # ---- main loop over query groups ----
    for g in range(NG):
        if 0 < g < NG - 1:
            dma_x(g + 1, list(range(8)), p_xt)


        # fill supply for this group's attention (deadline-ordered):
        #   [g=0 only: v0 + the rest of qkv0], qkv(g+1) q/k + v(g+1).
        # proj has no deadline before the kernel end, so ALL proj chains
        # are deferred to attn3 (the group with by far the most ACT-paced
        # attention to fill) — minus a few reserved for the kernel tail to
        # keep the PE busy (and HAM warm) through the last normalize.
        if g == 0:
            for tb in range(4):
                fill.append(v_chain(0, tb))
            for m in (1, 5, 2, 6, 3, 7):
                fill.append(qk_chain(0, m))
        if g + 1 < NG:
            for m in (0, 4, 1, 5, 2, 6, 3, 7):
                fill.append(qk_chain(g + 1, m))
            if g + 1 < NG - 1:
                for tb in range(4):
                    fill.append(v_chain(g + 1, tb))
        if g == NG - 1:
            for tb in range(4):
                fill.append(v_chain(g, tb))
            for gp in (0, 1, 2):
                for m in range(8):
                    fill.append(proj_chain(gp, m))
            tail_reserve = fill[-7:]
            del fill[-7:]

        K_g = 4 * (g + 1)
        # fill rate in CHAINS per attention block
        rate = {0: 1.45, 1: 0.4, 2: 0.18, 3: 0.42}[g]
        budget = 0.0
        with nc.named_scope(f"attn{g}"):
            yt_g = [p_yt.tile([128, QG], CDT, tag="yt", name=f"yt{g}_{c}")
                    for c in range(4)]
            yt_gs[g] = yt_g
            for hp in range(4):
                if g == 0 and hp == 1:
                    # group-0 prefetch DMAs deferred past hp0 so their
                    # descriptor generation stays off the startup critical
                    # path on the sync engine.
                    dma_x(1, list(range(8)), p_xt)
                    wp_t = p_wp.tile([128, 4, C], CDT, tag="wp")
                    nc.sync.dma_start(out=wp_t, in_=wpt[:, :, :])
                    for kc in range(4):
                        wp_sb.append(wp_t[:, kc, :])
                # the in-order PE queue deadlocks on any backward
                # dependency: this hp's q/k chains must be fully EMITTED
                # before its first S matmul (only group 0 has the
                # intra-group deadline).
                while ((g, hp) not in chains_done
                       or (g, 4 + hp) not in chains_done):
                    fill.pop(0)()
                psy = [ps_y.tile([128, QG], F32, tag="psy",
                                 name=f"psy{g}_{hp}_{r}") for r in range(2)]
                # software pipeline: emit S(kb), fill, AV(kb-1)
                es_q = []   # (kb, es tile)

                def emit_S(kb, hp=hp, g=g):
                    j = kb - 4 * g
                    c0 = max(0, 128 * j)
                    vis = slice(c0, QG)
                    ps = ps_s.tile([128, 2, QG], F32, tag="pss",
                                   name=f"pss{g}_{hp}_{kb}")
                    for r in (0, 1):
                        row = slice(64 * r, 64 * r + 64)
                        nc.tensor.matmul(
                            ps[:, r, vis],
                            k_sb[hp][row, kb * 128:(kb + 1) * 128],
                            q_gs[g][hp][row, vis], start=True, stop=True,
                            skip_group_check=True)
                    es = p_es.tile([128, 2, QG], ADT, tag="es")
                    nc.scalar.activation(out=es[:, :, vis], in_=ps[:, :, vis],
                                         func=EXP, scale=SCALE)
                    if j >= 0:
                        for r in (0, 1):
                            nc.vector.tensor_mul(es[:, r, c0:c0 + 128],
                                                 es[:, r, c0:c0 + 128],
                                                 tri_sb)
                    es_q.append((kb, es))

                def emit_AV(hp=hp, g=g, K_g=K_g):
                    kb, es = es_q.pop(0)
                    j = kb - 4 * g
                    c0 = max(0, 128 * j)
                    vis = slice(c0, QG)
                    # the v chain producing vp_sb[kb] must already be
                    # emitted (in-order PE queue): force-drain fill if not
                    while len(vp_sb) <= kb:
                        fill.pop(0)()
                    for r in (0, 1):
                        h = 2 * hp + r
                        nc.tensor.matmul(psy[r][0:65, vis],
                                         vp_sb[kb][:, h, :],
                                         es[:, r, vis], start=kb == 0,
                                         stop=kb == K_g - 1,
                                         skip_group_check=True)

                # stride-2 software pipeline: [S(b), S(b+1)] [fill]
                # [AV(b-2), AV(b-1)] — S leads AV by two blocks so the AVs
                # never wait on exp, and the pair batching halves the
                # per-insertion PE restart cost.
                for base in range(0, K_g, 2):
                    emit_S(base)
                    emit_S(base + 1)
                    budget += 2 * rate
                    while budget >= 1.0 and fill:
                        fill.pop(0)()
                        budget -= 1.0
                    while len(es_q) > 2:
                        emit_AV()
                while es_q:
                    emit_AV()

                tail = g == NG - 1 and hp == 3
                if tail:
                    # keep the PE busy (and HAM warm) through the final
                    # normalize chain
                    for ch in tail_reserve:
                        ch()
                # normalize off the PE critical path: psum -> sbuf, the two
                # heads' denominators batched into single Ln/Exp ACT ops,
                # DMA broadcast, scale on gpsimd (so the broadcast's DMA
                # roundtrip latency never head-of-line-blocks the DVE
                # queue, which gates chain psum reuse and the tri masks).
                ysbs = []
                sums = p_rec.tile([33, QG], F32R, tag="sums",
                                  name=f"sm{g}_{hp}")
                for r in (0, 1):
                    ysb = p_ysb.tile([65, QG], F32R, tag="ysb",
                                     name=f"ysb{g}_{hp}_{r}")
                    nc.vector.tensor_copy(out=ysb, in_=psy[r][0:65, :])
                    nc.vector.tensor_copy(out=sums[32 * r:32 * r + 1, :],
                                          in_=ysb[64:65, :])
                    ysbs.append(ysb)
                lns = p_rec.tile([33, QG], F32, tag="lns",
                                 name=f"ln{g}_{hp}")
                nc.scalar.activation(out=lns, in_=sums.bitcast(F32),
                                     func=mybir.ActivationFunctionType.Ln)
                rec = p_rec.tile([33, QG], F32R, tag="rec",
                                 name=f"rec{g}_{hp}")
                nc.scalar.activation(out=rec, in_=lns, func=EXP, scale=-1.0)
                if not tail:
                    bc = p_bc.tile([64, 2, QG], F32, tag="bc",
                                   name=f"bc{g}_{hp}")
                if tail:
                    # kernel tail: PE broadcast straight from rec, and the
                    # yt multiply reads the PSUM result directly — the DMA
                    # roundtrip latency would gate proj3
                    psbs = []
                    for r in (0, 1):
                        psb = ps_s.tile([128, 2, QG], F32, tag="pss",
                                        name=f"psbx{r}")
                        nc.tensor.matmul(
                            psb[0:64, 0, :],
                            ones_sb[32 * r:32 * r + 1, :],
                            rec[32 * r:32 * r + 1, :],
                            start=True, stop=True, skip_group_check=True)
                        psbs.append(psb)
                else:
                    recd = p_drb.tile([33, QG], F32, tag="recd",
                                      name=f"recd{g}_{hp}")
                    nc.sync.dma_start(out=recd, in_=rec.bitcast(F32))
                    # single broadcast DMA for both heads: row 32r -> 64 rows
                    nc.sync.dma_start(
                        out=bc,
                        in_=recd[0:33:32, :].rearrange(
                            "(o j) t -> o j t", o=1).to_broadcast([64, 2, QG]))
                for r in (0, 1):
                    mul_eng = nc.vector if (tail or MUL == "dve") else nc.gpsimd
                    mul_eng.tensor_mul(yt_g[hp][64 * r:64 * r + 64, :],
                                       ysbs[r][0:64, :].bitcast(F32),
                                       psbs[r][0:64, 0, :] if tail
                                       else bc[:, r, :])
        while fill:
            fill.pop(0)()

    with nc.named_scope("proj3"):
        for m in range(8):
            proj_chain(NG - 1, m)()


def _build_nc():
    from contextlib import ExitStack
    nc = bass.Bass(trn_type="TRN2")
    # all tensors host-pre-tiled so every DMA has >=2KB contiguous
    # per-partition runs
    xt = nc.dram_tensor("xt", [NG, 128, 8, QG], CDT, kind="ExternalInput")
    wqkt = nc.dram_tensor("wqkt", [4, 128, 2, 2 * CLOC], CDT,
                          kind="ExternalInput")
    wvt = nc.dram_tensor("wvt", [128, 8, CLOC], CDT, kind="ExternalInput")
    wpt = nc.dram_tensor("wpt", [128, 4, C], CDT, kind="ExternalInput")
    tri = nc.dram_tensor("tri", [KB, KB], CDT, kind="ExternalInput")
    ot = nc.dram_tensor("ot", [NG, 128, 8, QG], CDT, kind="ExternalOutput")
    with tile.TileContext(nc) as tc:
        with ExitStack() as ctx:
            _build_body(nc, tc, ctx, xt, wqkt, wvt, wpt, tri, ot)
    return nc


LAST_RESULTS = None
_NC_CACHE = None


def kernel(x, W_qkv, W_proj):
    global LAST_RESULTS, _NC_CACHE
    x = np.asarray(x, dtype=np.float32)
    W_qkv = np.asarray(W_qkv, dtype=np.float32)
    W_proj = np.asarray(W_proj, dtype=np.float32)

    if _NC_CACHE is None:
        _NC_CACHE = _build_nc()
    nc = _NC_CACHE
    import ml_dtypes
    tri = np.triu(np.ones((KB, KB), np.float32))
    in_maps = []
    for core in range(8):
        b, hg = core // 2, core % 2
        rq = slice(CLOC * hg, CLOC * hg + CLOC)
        Wq = W_qkv[0:C][rq]
        Wk = W_qkv[C:2 * C][rq]
        Wv = W_qkv[2 * C:3 * C][rq]
        # x[b].T is [C, T]: tile to [NG, 128, 8, QG] with
        # xt[g, p, kc, t] = x.T[kc*128+p, g*QG+t]
        xtb = x[b].T.reshape(8, 128, NG, QG).transpose(2, 1, 0, 3)
        wqk = np.concatenate([Wq, Wk], axis=0).T  # [C, 2*CLOC]
        wqk4 = wqk.reshape(4, 2, 128, 2 * CLOC).transpose(0, 2, 1, 3)
        wv8 = Wv.T.reshape(8, 128, CLOC).transpose(1, 0, 2)
        wp4 = W_proj[:, rq].T.reshape(4, 128, C).transpose(1, 0, 2)
        _c = lambda a: np.ascontiguousarray(a).astype(ml_dtypes.bfloat16)
        in_maps.append({
            "xt": _c(xtb),
            "wqkt": _c(wqk4),
            "wvt": _c(wv8),
            "wpt": _c(wp4),
            "tri": _c(tri),
        })

    trace = os.environ.get("ATTN_BASS_TRACE") == "1"
    res = None
    last_exc = None
    for attempt in range(3):
        try:
            res = run_bass_kernel_spmd(nc, in_maps, core_ids=list(range(8)),
                                       trace=trace)
            break
        except Exception as e:  # transient NRT device errors happen
            last_exc = e
            import time as _time
            _time.sleep(2.0)
    if res is None:
        raise last_exc
    LAST_RESULTS = res
    out = np.empty((B, T, C), np.float32)
    for b in range(B):
        acc = (res.results[2 * b]["ot"].astype(np.float32)
               + res.results[2 * b + 1]["ot"].astype(np.float32))
        # [NG, 128, 8, QG] -> [C, T] -> [T, C]
        out[b] = acc.transpose(2, 1, 0, 3).reshape(C, T).T
    return out


# revision 61
# speedup vs baseline: 1.0104x; 1.0104x over previous
"""Trainium2 Bass kernel for NanoAttention (B=4, T=2048, C=1024, H=16, causal).

Sharding: 8 cores = 4 batches x 2 head-groups (8 heads each).
Per core (b, hg):
  - column-parallel qkv:  q,k produced in [channel, token] (transposed) layout,
    v produced in [token, channel] (natural) layout with an appended ones
    column per head (fuses the softmax denominator into the AV matmul).
  - causal attention per head with S^T in [key, query] layout; exp on ACT with
    scale=1/sqrt(D); block-causal skipping + triangular masking on the
    diagonal blocks.
  - row-parallel proj producing a partial output [C, T] in bf16; the host adds
    the two head-group partials per batch and transposes back.

v2 structural changes vs v1 (319us):
  - software-pipelined emission: the PE queue is in-order, so S(k+1) is
    emitted BEFORE AV(k) and independent dense matmuls (qkv of the next
    group, proj of the previous, v chains) are drip-fed between attention
    blocks to fill the PE during ACT(exp)-paced stretches.
  - startup: xt/wqk DMAs interleaved per contraction chunk so the first
    matmul starts after ~256KB instead of ~3MB; keeps HAM at 8/8.
  - dense psum chains double-buffered (2 bufs) to kill inter-chain stalls.
  - softmax reciprocal on DVE (reciprocal_approx_fast) instead of ACT Ln/Exp.
  - reciprocal broadcast via gpsimd partition_broadcast (no DRAM roundtrip).
  - output DMA in bf16 (host accumulates partials in f32).
"""
import os
import sys

sys.path.insert(0, '/opt/trn_rl_repo')

import numpy as np
import orjson

import concourse.bass as bass
import concourse.mybir as mybir
import concourse.tile as tile
from concourse.bass_utils import run_bass_kernel_spmd

# ---------------------------------------------------------------------------
# Workaround for this container's walrus build: it enforces the HW limit of
# one sync-wait per instruction (two for EventSemaphore), but Tile's sem
# assignment can emit more (kernel-tail Drain waits on every DMA queue used;
# HWDGE stores can pick up two queue waits). Split the overflow onto
# preceding pure-wait EventSemaphore instructions on the same engine at
# JSON-serialization time so every compile path is covered.
# ---------------------------------------------------------------------------


def _split_multi_waits(data):
    n_split = 0
    for func in data.get("functions", []):
        for blk in func.get("blocks", []):
            insts = blk.get("instructions")
            if not insts:
                continue
            out = []
            for inst in insts:
                si = inst.get("sync_info")
                waits = (si or {}).get("on_wait") or []
                cap = 2 if inst.get("opcode") == "EventSemaphore" else 1
                if len(waits) > cap and "engine" in inst:
                    extra = waits[:-cap]
                    si["on_wait"] = waits[-cap:]
                    for i in range(0, len(extra), 2):
                        n_split += 1
                        out.append({
                            "debug": inst.get("debug"),
                            "engine": inst["engine"],
                            "ins": [],
                            "outs": [],
                            "name": f"{inst['name']}_wsplit{n_split}",
                            "opcode": "EventSemaphore",
                            "sync_info": {"on_wait": extra[i:i + 2],
                                          "on_update": []},
                        })
                out.append(inst)
            blk["instructions"] = out
    return data


_orig_to_json_bytes = bass.Bass.to_json_bytes


def _patched_to_json_bytes(self):
    return orjson.dumps(_split_multi_waits(orjson.loads(_orig_to_json_bytes(self))))


bass.Bass.to_json_bytes = _patched_to_json_bytes

# ---------------------------------------------------------------------------

B, T, C = 4, 2048, 1024
N_HEAD, D = 16, 64
HLOC = 8          # heads per core
CLOC = HLOC * D   # 512 local qkv channels per core
QG = 512          # query-group width
NG = T // QG      # 4 query groups
KB = 128          # key-block width
F32R = mybir.dt.float32r
F32 = mybir.dt.float32
BF16 = mybir.dt.bfloat16
CDT = BF16
ADT = CDT
EXP = mybir.ActivationFunctionType.Exp
SCALE = 1.0 / np.sqrt(D)
# reciprocal broadcast path: "dma" (DRAM roundtrip broadcast) or "gpsimd"
# (partition_broadcast — does NOT compile in this container's walrus:
# "ISA wrong length", same for the custom-DVE reciprocal_approx_fast).
BCAST = os.environ.get("ATTN_BCAST", "dma")
# engine for the yt normalize multiplies: "gpsimd" keeps the DMA-broadcast
# wait out of the DVE queue; "dve" is the fallback.
MUL = os.environ.get("ATTN_MUL", "gpsimd")


def _build_body(nc, tc, ctx, xt, wqkt, wvt, wpt, tri, ot):
    p_wqk = ctx.enter_context(tc.tile_pool(name="wqk", bufs=4))
    p_wv = ctx.enter_context(tc.tile_pool(name="wv", bufs=1))
    p_wp = ctx.enter_context(tc.tile_pool(name="wp", bufs=1))
    p_xt0 = ctx.enter_context(tc.tile_pool(name="xt0", bufs=4))
    p_xt = ctx.enter_context(tc.tile_pool(name="xt", bufs=2))
    p_k = ctx.enter_context(tc.tile_pool(name="ksb", bufs=4))
    p_q = ctx.enter_context(tc.tile_pool(name="qsb", bufs=8))
    p_vp = ctx.enter_context(tc.tile_pool(name="vp", bufs=16))
    p_es = ctx.enter_context(tc.tile_pool(name="es", bufs=7))
    # all four groups' yt stay alive: proj is deferred to attn3/tail
    p_yt = ctx.enter_context(tc.tile_pool(name="yt", bufs=16))
    p_ost = ctx.enter_context(tc.tile_pool(name="ost", bufs=4))
    p_one = ctx.enter_context(tc.tile_pool(name="one", bufs=1))
    p_rec = ctx.enter_context(tc.tile_pool(name="rec", bufs=6))
    p_ysb = ctx.enter_context(tc.tile_pool(name="ysb", bufs=6))
    p_bc = ctx.enter_context(tc.tile_pool(name="bc", bufs=4))
    if BCAST == "dma":
        p_drb = ctx.enter_context(tc.tile_pool(name="drb", bufs=2, space="DRAM"))
    ps_mm = ctx.enter_context(tc.tile_pool(name="psmm", bufs=2, space="PSUM"))
    ps_s = ctx.enter_context(tc.tile_pool(name="pss", bufs=2, space="PSUM"))
    ps_y = ctx.enter_context(tc.tile_pool(name="psy", bufs=2, space="PSUM"))

    # ---- static state ----
    # Each dma_start costs ~630ns of serial descriptor-generation on the
    # sync engine, so inputs are loaded with FEW multi-chunk transfers:
    # contraction-chunk views are AP slices of [128, n, free] tiles.
    wqk_sb = [None] * 8   # views: wqk_sb[kc] = [128, 2*CLOC]
    wv_sb = [None] * 8
    wp_sb = []
    xt_g = [[None] * 8 for _ in range(NG)]
    tri_sb = p_one.tile([KB, KB], CDT, tag="tri")
    ones_sb = p_one.tile([33, 64], F32R, tag="ones")
    nc.vector.memset(ones_sb.bitcast(F32), 1.0)
    k_sb = [p_k.tile([128, T], ADT, tag="ksb", name=f"ksb{c}") for c in range(4)]
    q_gs = [[None] * 4 for _ in range(NG)]
    vp_sb = []        # grows to 16 tiles, 4 per group
    yt_gs = [None] * NG

    def dma_x(g, kcs, pool):
        # one dma_start covering contraction chunks kcs (list); the host
        # pre-tiles x as [NG, 128, 8, QG] so each partition's run is
        # contiguous (n*1KB descriptors instead of n*128)
        n = len(kcs)
        t = pool.tile([128, n, QG], CDT, tag="xt", name=f"xt{g}_{kcs[0]}")
        nc.sync.dma_start(out=t, in_=xt[g, :, kcs[0]:kcs[0] + n, :])
        for i, kc in enumerate(kcs):
            xt_g[g][kc] = t[:, i, :]

    def dma_wqk(kc0):
        # one dma_start per PAIR of contraction chunks (host layout
        # [4, 128, 2, 2*CLOC])
        t = p_wqk.tile([128, 2, 2 * CLOC], CDT, tag="wqk", name=f"wqk{kc0}")
        nc.sync.dma_start(out=t, in_=wqkt[kc0 // 2, :, :, :])
        wqk_sb[kc0] = t[:, 0, :]
        wqk_sb[kc0 + 1] = t[:, 1, :]

    # ---- dense-fill machinery: each fill item emits ONE FULL accumulation
    # chain (8 or 4 back-to-back matmuls + epilogue). Chain granularity
    # keeps the PE instruction stream uniform: scattering single matmuls
    # between attention ops costs a ~130ns restart per insertion. ----
    def qk_chain(g, m):
        # m in 0..7: 0..3 -> q chunks (hp=m), 4..7 -> k chunks (hp=m-4)
        def emit(g=g, m=m):
            ps = ps_mm.tile([128, QG], F32, tag="psmm", name=f"qk{g}_{m}")
            for kc in range(8):
                nc.tensor.matmul(ps, wqk_sb[kc][:, m * 128:(m + 1) * 128],
                                 xt_g[g][kc], start=kc == 0, stop=kc == 7,
                                 skip_group_check=True)
            if m < 4:
                qt = p_q.tile([128, QG], ADT, tag="qsb", name=f"q{g}_{m}")
                nc.vector.tensor_copy(out=qt, in_=ps)
                q_gs[g][m] = qt
            else:
                nc.vector.tensor_copy(
                    out=k_sb[m - 4][:, g * QG:(g + 1) * QG], in_=ps)
            chains_done.add((g, m))
        return emit

    def v_chain(g, tb):
        def emit(g=g, tb=tb):
            ps = ps_mm.tile([128, CLOC], F32, tag="psmm", name=f"v{g}_{tb}")
            for kc in range(8):
                nc.tensor.matmul(ps, xt_g[g][kc][:, tb * 128:(tb + 1) * 128],
                                 wv_sb[kc], start=kc == 0, stop=kc == 7,
                                 skip_group_check=True)
            vp = p_vp.tile([128, HLOC, 65], ADT, tag="vp", name=f"vp{g}_{tb}")
            nc.vector.memset(vp[:, :, 64:65], 1.0)
            nc.vector.tensor_copy(
                out=vp[:, :, 0:64], in_=ps.rearrange("p (h d) -> p h d", d=64))
            vp_sb.append(vp)
        return emit

    ost_pending = {}

    def proj_chain(g, m):
        def emit(g=g, m=m):
            ps = ps_mm.tile([128, QG], F32, tag="psmm", name=f"pj{g}_{m}")
            for c in range(4):
                nc.tensor.matmul(ps, wp_sb[c][:, m * 128:(m + 1) * 128],
                                 yt_gs[g][c], start=c == 0, stop=c == 3,
                                 skip_group_check=True)
            # pair the output stores: one dma_start per two m-chunks
            # (halves the serial descriptor-generation on the sync engine)
            if m % 2 == 0:
                ost = p_ost.tile([128, 2, QG], CDT, tag="ost",
                                 name=f"ost{g}_{m}")
                ost_pending[g] = ost
            else:
                ost = ost_pending[g]
            nc.vector.tensor_copy(out=ost[:, m % 2, :], in_=ps)
            if m % 2 == 1:
                nc.sync.dma_start(out=ot[g, :, m - 1:m + 1, :], in_=ost)
        return emit

    fill = []          # FIFO of pending dense chains
    chains_done = set()

    # ---- PE pre-warm: the HAM clock gate starts at 4/8 (1.2 GHz) and
    # needs ~3.4us of sustained PE busy to release. The first real matmul
    # can't start before ~13us (NEFF init + first DMAs), so burn dummy
    # matmuls that depend only on a memset — the PE is warm before the
    # first chain. ----
    dmy = p_one.tile([1, QG], F32R, tag="dmy")
    nc.vector.memset(dmy.bitcast(F32), 1.0)
    for i in range(18):
        psd = ps_s.tile([128, 2, QG], F32, tag="pss", name=f"warm{i}")
        nc.tensor.matmul(psd[0:64, 0, :], ones_sb[0:1, :], dmy,
                         start=True, stop=True, skip_group_check=True)

    # ---- startup: interleaved wqk/xt0 pair-DMAs ordered by first
    # consumption, wv early (v chains are the first fill); first chains
    # ASAP ----
    for kc0 in range(0, 8, 2):
        dma_wqk(kc0)
        dma_x(0, [kc0, kc0 + 1], p_xt0)
        if kc0 == 2:
            wv_t = p_wv.tile([128, 8, CLOC], CDT, tag="wv")
            nc.sync.dma_start(out=wv_t, in_=wvt[:, :, :])
            for kc in range(8):
                wv_sb[kc] = wv_t[:, kc, :]
    nc.sync.dma_start(out=tri_sb, in_=tri[:, :])

    with nc.named_scope("qkv0"):
        qk_chain(0, 0)()
        qk_chain(0, 4)()

    # ---- main loop over query groups ----
    for g in range(NG):
        if 0 < g < NG - 1:
            dma_x(g + 1, list(range(8)), p_xt)


        # fill supply for this group's attention (deadline-ordered):
        #   [g=0 only: v0 + the rest of qkv0], qkv(g+1) q/k + v(g+1).
        # proj has no deadline before the kernel end, so ALL proj chains
        # are deferred to attn3 (the group with by far the most ACT-paced
        # attention to fill) — minus a few reserved for the kernel tail to
        # keep the PE busy (and HAM warm) through the last normalize.
        if g == 0:
            for tb in range(4):
                fill.append(v_chain(0, tb))
            for m in (1, 5, 2, 6, 3, 7):
                fill.append(qk_chain(0, m))
        if g + 1 < NG:
            for m in (0, 4, 1, 5, 2, 6, 3, 7):
                fill.append(qk_chain(g + 1, m))
            if g + 1 < NG - 1:
                for tb in range(4):
                    fill.append(v_chain(g + 1, tb))
        if g == NG - 1:
            for tb in range(4):
                fill.append(v_chain(g, tb))
            for gp in (0, 1, 2):
                for m in range(8):
                    fill.append(proj_chain(gp, m))
            tail_reserve = fill[-7:]
            del fill[-7:]

        K_g = 4 * (g + 1)
        # fill rate in CHAINS per attention block
        rate = {0: 1.45, 1: 0.4, 2: 0.18, 3: 0.42}[g]
        budget = 0.0
        with nc.named_scope(f"attn{g}"):
            yt_g = [p_yt.tile([128, QG], CDT, tag="yt", name=f"yt{g}_{c}")
                    for c in range(4)]
            yt_gs[g] = yt_g
            for hp in range(4):
                if g == 0 and hp == 1:
                    # group-0 prefetch DMAs deferred past hp0 so their
                    # descriptor generation stays off the startup critical
                    # path on the sync engine.
                    dma_x(1, list(range(8)), p_xt)
                    wp_t = p_wp.tile([128, 4, C], CDT, tag="wp")
                    nc.sync.dma_start(out=wp_t, in_=wpt[:, :, :])
                    for kc in range(4):
                        wp_sb.append(wp_t[:, kc, :])
                # the in-order PE queue deadlocks on any backward
                # dependency: this hp's q/k chains must be fully EMITTED
                # before its first S matmul (only group 0 has the
                # intra-group deadline).
                while ((g, hp) not in chains_done
                       or (g, 4 + hp) not in chains_done):
                    fill.pop(0)()
                psy = [ps_y.tile([128, QG], F32, tag="psy",
                                 name=f"psy{g}_{hp}_{r}") for r in range(2)]
                # software pipeline: emit S(kb), fill, AV(kb-1)
                es_q = []   # (kb, es tile)

                def emit_S(kb, hp=hp, g=g):
                    j = kb - 4 * g
                    c0 = max(0, 128 * j)
                    vis = slice(c0, QG)
                    ps = ps_s.tile([128, 2, QG], F32, tag="pss",
                                   name=f"pss{g}_{hp}_{kb}")
                    for r in (0, 1):
                        row = slice(64 * r, 64 * r + 64)
                        nc.tensor.matmul(
                            ps[:, r, vis],
                            k_sb[hp][row, kb * 128:(kb + 1) * 128],
                            q_gs[g][hp][row, vis], start=True, stop=True,
                            skip_group_check=True)
                    es = p_es.tile([128, 2, QG], ADT, tag="es")
                    nc.scalar.activation(out=es[:, :, vis], in_=ps[:, :, vis],
                                         func=EXP, scale=SCALE)
                    if j >= 0:
                        for r in (0, 1):
                            nc.vector.tensor_mul(es[:, r, c0:c0 + 128],
                                                 es[:, r, c0:c0 + 128],
                                                 tri_sb)
                    es_q.append((kb, es))

                def emit_AV(hp=hp, g=g, K_g=K_g):
                    kb, es = es_q.pop(0)
                    j = kb - 4 * g
                    c0 = max(0, 128 * j)
                    vis = slice(c0, QG)
                    # the v chain producing vp_sb[kb] must already be
                    # emitted (in-order PE queue): force-drain fill if not
                    while len(vp_sb) <= kb:
                        fill.pop(0)()
                    for r in (0, 1):
                        h = 2 * hp + r
                        nc.tensor.matmul(psy[r][0:65, vis],
                                         vp_sb[kb][:, h, :],
                                         es[:, r, vis], start=kb == 0,
                                         stop=kb == K_g - 1,
                                         skip_group_check=True)

                # stride-2 software pipeline: [S(b), S(b+1)] [fill]
                # [AV(b-2), AV(b-1)] — S leads AV by two blocks so the AVs
                # never wait on exp, and the pair batching halves the
                # per-insertion PE restart cost.
                for base in range(0, K_g, 2):
                    emit_S(base)
                    emit_S(base + 1)
                    budget += 2 * rate
                    while budget >= 1.0 and fill:
                        fill.pop(0)()
                        budget -= 1.0
                    while len(es_q) > 2:
                        emit_AV()
                while es_q:
                    emit_AV()

                tail = g == NG - 1 and hp == 3
                if tail:
                    # keep the PE busy (and HAM warm) through the final
                    # normalize chain
                    for ch in tail_reserve:
                        ch()
                # normalize off the PE critical path: psum -> sbuf, the two
                # heads' denominators batched into single Ln/Exp ACT ops,
                # DMA broadcast, scale on gpsimd (so the broadcast's DMA
                # roundtrip latency never head-of-line-blocks the DVE
                # queue, which gates chain psum reuse and the tri masks).
                ysbs = []
                sums = p_rec.tile([33, QG], F32R, tag="sums",
                                  name=f"sm{g}_{hp}")
                for r in (0, 1):
                    ysb = p_ysb.tile([65, QG], F32R, tag="ysb",
                                     name=f"ysb{g}_{hp}_{r}")
                    nc.vector.tensor_copy(out=ysb, in_=psy[r][0:65, :])
                    nc.vector.tensor_copy(out=sums[32 * r:32 * r + 1, :],
                                          in_=ysb[64:65, :])
                    ysbs.append(ysb)
                lns = p_rec.tile([33, QG], F32, tag="lns",
                                 name=f"ln{g}_{hp}")
                nc.scalar.activation(out=lns, in_=sums.bitcast(F32),
                                     func=mybir.ActivationFunctionType.Ln)
                rec = p_rec.tile([33, QG], F32R, tag="rec",
                                 name=f"rec{g}_{hp}")
                nc.scalar.activation(out=rec, in_=lns, func=EXP, scale=-1.0)
                if not tail:
                    bc = p_bc.tile([64, 2, QG], F32, tag="bc",
                                   name=f"bc{g}_{hp}")
                if tail:
                    # kernel tail: PE broadcast straight from rec, and the
                    # yt multiply reads the PSUM result directly — the DMA
                    # roundtrip latency would gate proj3
                    psbs = []
                    for r in (0, 1):
                        psb = ps_s.tile([128, 2, QG], F32, tag="pss",
                                        name=f"psbx{r}")
                        nc.tensor.matmul(
                            psb[0:64, 0, :],
                            ones_sb[32 * r:32 * r + 1, :],
                            rec[32 * r:32 * r + 1, :],
                            start=True, stop=True, skip_group_check=True)
                        psbs.append(psb)
                else:
                    recd = p_drb.tile([33, QG], F32, tag="recd",
                                      name=f"recd{g}_{hp}")
                    nc.sync.dma_start(out=recd, in_=rec.bitcast(F32))
                    # single broadcast DMA for both heads: row 32r -> 64 rows
                    nc.sync.dma_start(
                        out=bc,
                        in_=recd[0:33:32, :].rearrange(
                            "(o j) t -> o j t", o=1).to_broadcast([64, 2, QG]))
                for r in (0, 1):
                    mul_eng = nc.vector if (tail or MUL == "dve") else nc.gpsimd
                    mul_eng.tensor_mul(yt_g[hp][64 * r:64 * r + 64, :],
                                       ysbs[r][0:64, :].bitcast(F32),
                                       psbs[r][0:64, 0, :] if tail
                                       else bc[:, r, :])
        while fill:
            fill.pop(0)()

    with nc.named_scope("proj3"):
        for m in range(8):
            proj_chain(NG - 1, m)()


def _build_nc():
    from contextlib import ExitStack
    nc = bass.Bass(trn_type="TRN2")
    # all tensors host-pre-tiled so every DMA has >=2KB contiguous
    # per-partition runs
    xt = nc.dram_tensor("xt", [NG, 128, 8, QG], CDT, kind="ExternalInput")
    wqkt = nc.dram_tensor("wqkt", [4, 128, 2, 2 * CLOC], CDT,
                          kind="ExternalInput")
    wvt = nc.dram_tensor("wvt", [128, 8, CLOC], CDT, kind="ExternalInput")
    wpt = nc.dram_tensor("wpt", [128, 4, C], CDT, kind="ExternalInput")
    tri = nc.dram_tensor("tri", [KB, KB], CDT, kind="ExternalInput")
    ot = nc.dram_tensor("ot", [NG, 128, 8, QG], CDT, kind="ExternalOutput")
    with tile.TileContext(nc) as tc:
        with ExitStack() as ctx:
            _build_body(nc, tc, ctx, xt, wqkt, wvt, wpt, tri, ot)
    return nc


LAST_RESULTS = None
_NC_CACHE = None


def kernel(x, W_qkv, W_proj):
    global LAST_RESULTS, _NC_CACHE
    x = np.asarray(x, dtype=np.float32)
    W_qkv = np.asarray(W_qkv, dtype=np.float32)
    W_proj = np.asarray(W_proj, dtype=np.float32)

    if _NC_CACHE is None:
        _NC_CACHE = _build_nc()
    nc = _NC_CACHE
    import ml_dtypes
    tri = np.triu(np.ones((KB, KB), np.float32))
    in_maps = []
    for core in range(8):
        b, hg = core // 2, core % 2
        rq = slice(CLOC * hg, CLOC * hg + CLOC)
        Wq = W_qkv[0:C][rq]
        Wk = W_qkv[C:2 * C][rq]
        Wv = W_qkv[2 * C:3 * C][rq]
        # x[b].T is [C, T]: tile to [NG, 128, 8, QG] with
        # xt[g, p, kc, t] = x.T[kc*128+p, g*QG+t]
        xtb = x[b].T.reshape(8, 128, NG, QG).transpose(2, 1, 0, 3)
        wqk = np.concatenate([Wq, Wk], axis=0).T  # [C, 2*CLOC]
        wqk4 = wqk.reshape(4, 2, 128, 2 * CLOC).transpose(0, 2, 1, 3)
        wv8 = Wv.T.reshape(8, 128, CLOC).transpose(1, 0, 2)
        wp4 = W_proj[:, rq].T.reshape(4, 128, C).transpose(1, 0, 2)
        _c = lambda a: np.ascontiguousarray(a).astype(ml_dtypes.bfloat16)
        in_maps.append({
            "xt": _c(xtb),
            "wqkt": _c(wqk4),
            "wvt": _c(wv8),
            "wpt": _c(wp4),
            "tri": _c(tri),
        })

    trace = os.environ.get("ATTN_BASS_TRACE") == "1"
    res = None
    last_exc = None
    for attempt in range(3):
        try:
            res = run_bass_kernel_spmd(nc, in_maps, core_ids=list(range(8)),
                                       trace=trace)
            break
        except Exception as e:  # transient NRT device errors happen
            last_exc = e
            import time as _time
            _time.sleep(2.0)
    if res is None:
        raise last_exc
    LAST_RESULTS = res
    out = np.empty((B, T, C), np.float32)
    for b in range(B):
        acc = (res.results[2 * b]["ot"].astype(np.float32)
               + res.results[2 * b + 1]["ot"].astype(np.float32))
        # [NG, 128, 8, QG] -> [C, T] -> [T, C]
        out[b] = acc.transpose(2, 1, 0, 3).reshape(C, T).T
    return out
